# revision 32
# baseline (speedup 1.0000x reference)
"""Multi-head attention layer on 8 TRN2 NeuronCores.

Sharding: core c handles batch b = c // 4 and head group hg = c % 4
(4 heads, organized as 2 pairs). Each core computes its heads' QKV
projection, attention, and a row-parallel slice of the output
projection; the host sums the 4 partial outputs per batch and adds the
biases that commute with the projection (proj_b and the v-bias term).

Device schedule (per core, bf16 matmuls, fp32 PSUM accumulation):
  emission order is q0/k0, v, pair-0 attention (boosted to top
  scheduler priority so ScalarE's exp stream — the bottleneck engine —
  starts as early as deps allow), q1/k1, pair-1 attention with the
  output projection interleaved one query-block behind. attn@v matmuls
  lag the scores/exp stream by 5 j-tiles so slot waits never stall the
  in-order PE queue.

  qT/kT   4 x [128, 512] per pair (f on partitions, 2 heads stacked;
          one tile per 512-l block so attention starts after the first
          q/k blocks instead of the whole projection)
  v_aug   16 x [128, 272]: v natural (l on partitions), per-head 68-col
          blocks [v(64) | ones(1) | pad(3)]
  scores  sT[j, i] per (pair, i-block, j-tile) in one [128, 1024] f32
          PSUM tile; head A cols 0:512, head B 512:1024; the two
          matmuls use disjoint PE row groups (row tiling) so they
          overlap on hardware
  exp     one ACT Exp per scores tile (FD=1024), scale=0.125, -> bf16
  out^T   [65, 512] fp32 PSUM accum over j-tiles per head; row 64 =
          softmax denominator (ones column of v_aug)
  norm    DVE reciprocal -> gpsimd partition_broadcast -> DVE mul
  proj    out[l, dout] partial = attn^T.T @ projwT, fp32 out
"""

import numpy as np
import ml_dtypes

import concourse.bass as bass
import concourse.mybir as mybir
import concourse.tile as tile
from concourse import bacc
from concourse.bass_utils import run_bass_kernel_spmd

N_CORES = 8
B, L, D = 2, 2048, 1024
H, HD = 16, 64
SCALE = HD ** (-0.5)
HPC = 4  # heads per core
BF16 = mybir.dt.bfloat16
F32 = mybir.dt.float32
Exp = mybir.ActivationFunctionType.Exp

N_DT = D // 128        # 8 d-tiles (contraction for qkv)
N_LB = L // 512        # 4 l-blocks of 512
N_LT = L // 128        # 16 l-tiles of 128
N_JT = L // 128        # 16 key tiles
N_IB = L // 512        # 4 query blocks of 512
VW = 68                # per-head stride in v_aug: 64 v + 1 ones + 3 pad


def build_nc():
    nc = bacc.Bacc(None, target_bir_lowering=False, debug=False)

    xT_d = nc.dram_tensor("xT", [D, L], BF16, kind="ExternalInput")
    wq_d = nc.dram_tensor("wqkvT", [D, 768], BF16, kind="ExternalInput")
    bias_d = nc.dram_tensor("bias_qk", [128, 4], F32, kind="ExternalInput")
    pw_d = nc.dram_tensor("projwT", [256, D], BF16, kind="ExternalInput")
    out_d = nc.dram_tensor("out", [L, D], F32, kind="ExternalOutput")

    with tile.TileContext(nc) as tc:
        with (
            tc.tile_pool(name="persist", bufs=1) as pp,
            tc.tile_pool(name="work", bufs=2) as wp,
            tc.tile_pool(name="ps_sc", bufs=2, space="PSUM") as ps_sc,
            tc.tile_pool(name="ps_acc", bufs=2, space="PSUM") as ps_acc,
            tc.tile_pool(name="ps_mm", bufs=2, space="PSUM") as ps_mm,
        ):
            # ---- persistent inputs. DMA order: bias, weights, then x in
            # per-512 column chunks (lb0 first) so the first q/k
            # accumulation groups complete ~8us in instead of ~15us.
            bias_sb = pp.tile([128, 4], F32, tag="bias", name="bias")
            nc.gpsimd.dma_start(out=bias_sb[:], in_=bias_d[:])
            wq = []
            for dt in range(N_DT):
                tw = pp.tile([128, 768], BF16, tag=f"wq{dt}", name=f"wq{dt}")
                # weights split across two otherwise-idle SWDGE queues so
                # they land in parallel with the x chunks on the sync queue
                eng = nc.scalar if dt < 4 else nc.gpsimd
                eng.dma_start(out=tw[:], in_=wq_d[dt * 128:(dt + 1) * 128, :])
                wq.append(tw)
            xtc = [[pp.tile([128, 512], BF16, tag=f"xt{dt}{lb}", name=f"xt{dt}{lb}")
                    for lb in range(N_LB)] for dt in range(N_DT)]
            for lb in range(N_LB):
                for dt in range(N_DT):
                    nc.sync.dma_start(
                        out=xtc[dt][lb][:],
                        in_=xT_d[dt * 128:(dt + 1) * 128,
                                 lb * 512:(lb + 1) * 512],
                    )

            # qT/kT: one tile per 512-l block for fine-grained scheduling
            qT = [[pp.tile([128, 512], BF16, tag=f"q{p}{lb}", name=f"q{p}{lb}")
                   for lb in range(N_LB)] for p in range(2)]
            kT = [[pp.tile([128, 512], BF16, tag=f"k{p}{lb}", name=f"k{p}{lb}")
                   for lb in range(N_LB)] for p in range(2)]
            va = [pp.tile([128, HPC * VW], BF16, tag=f"va{lt}", name=f"va{lt}")
                  for lt in range(N_LT)]
            attn = [[pp.tile([128, 512], BF16, tag=f"at{p}{ib}", name=f"at{p}{ib}")
                     for ib in range(N_IB)] for p in range(2)]
            # ones columns of v_aug first: they gate attn@v ~15us in,
            # while the projection weights behind them are not needed
            # until the pair-1 phase
            for lt in range(N_LT):
                for blk in range(HPC):
                    nc.gpsimd.memset(va[lt][:, blk * VW + 64: blk * VW + 65], 1.0)
            projw = []
            for fk in range(2):
                t = pp.tile([128, D], BF16, tag=f"pw{fk}", name=f"pw{fk}")
                nc.gpsimd.dma_start(out=t[:], in_=pw_d[fk * 128:(fk + 1) * 128, :])
                projw.append(t)

            def qk_proj(p):
                """qT/kT for pair p: [f, l] orientation, f on partitions."""
                for lb in range(N_LB):
                    lsl = slice(lb * 512, (lb + 1) * 512)
                    for g, dest in ((p, qT[p][lb]), (2 + p, kT[p][lb])):
                        ps = ps_mm.tile([128, 512], F32, tag="mm", name="psqk")
                        for dt in range(N_DT):
                            nc.tensor.matmul(
                                ps[:],
                                wq[dt][:, g * 128:(g + 1) * 128],
                                xtc[dt][lb][:],
                                start=(dt == 0),
                                stop=(dt == N_DT - 1),
                            )
                        nc.vector.tensor_scalar_add(
                            dest[:], ps[:], bias_sb[:, g:g + 1]
                        )

            def v_proj():
                """v natural [l, f] into v_aug blocks (no bias; host-folded)."""
                for lt in range(N_LT):
                    ps = ps_mm.tile([128, 256], F32, tag="mm", name="psv")
                    for dt in range(N_DT):
                        nc.tensor.matmul(
                            ps[:],
                            xtc[dt][lt // 4][:, (lt % 4) * 128:(lt % 4) * 128 + 128],
                            wq[dt][:, 512:768],
                            start=(dt == 0),
                            stop=(dt == N_DT - 1),
                        )
                    # one strided-dest copy for all 4 head blocks
                    # (dest stride 68 bf16 = 136 B, 4B-aligned)
                    dst = va[lt][:, 0:HPC * VW].rearrange(
                        "p (b w) -> p b w", b=HPC
                    )[:, :, 0:64]
                    srcv = ps[:, 0:256].rearrange("p (b w) -> p b w", b=HPC)
                    nc.vector.tensor_copy(dst, srcv)

            def proj_unit(ib, lq, db):
                lsl = slice(lq * 128, (lq + 1) * 128)
                lt = ib * 4 + lq
                dsl = slice(db * 512, (db + 1) * 512)
                ps = ps_mm.tile([128, 512], F32, tag="mm", name="pspj")
                for fk in range(2):
                    nc.tensor.matmul(
                        ps[:], attn[fk][ib][:, lsl], projw[fk][:, dsl],
                        start=(fk == 0), stop=(fk == 1),
                    )
                osb = wp.tile([128, 512], F32, tag="osb", name="osb", bufs=6)
                nc.vector.tensor_copy(osb[:], ps[:])
                nc.sync.dma_start(
                    out=out_d[lt * 128:(lt + 1) * 128, dsl], in_=osb[:])

            def proj_block(ib):
                for lq in range(4):
                    lsl = slice(lq * 128, (lq + 1) * 128)
                    lt = ib * 4 + lq
                    for db in range(2):
                        dsl = slice(db * 512, (db + 1) * 512)
                        ps = ps_mm.tile([128, 512], F32, tag="mm",
                                        name="pspj")
                        for fk in range(2):
                            nc.tensor.matmul(
                                ps[:], attn[fk][ib][:, lsl],
                                projw[fk][:, dsl],
                                start=(fk == 0), stop=(fk == 1),
                            )
                        osb = wp.tile([128, 512], F32, tag="osb",
                                      name="osb", bufs=6)
                        nc.vector.tensor_copy(osb[:], ps[:])
                        nc.sync.dma_start(
                            out=out_d[lt * 128:(lt + 1) * 128, dsl],
                            in_=osb[:])

            def attention(p, with_proj):
                for ib in range(N_IB):
                    acc = [
                        ps_acc.tile([65, 512], F32, tag="acc", name="acc")
                        for _ in range(2)
                    ]
                    # attn@v lags the scores/exp stream by LAG j-tiles so
                    # its wait for an acc slot (previous block's norm) does
                    # not stall the in-order PE queue at block transitions
                    LAG = 5
                    exs = {}
                    hp = tc.high_priority() if p == 0 else None
                    if hp:
                        hp.__enter__()
                    for jtt in range(N_JT + LAG):
                        if jtt < N_JT:
                            jt = jtt
                            sc = ps_sc.tile([128, 1024], F32, tag="sc", name="sc")
                            jsl = slice((jt % 4) * 128, (jt % 4) * 128 + 128)
                            kt = kT[p][jt // 4]
                            qt = qT[p][ib]
                            # pair-packed scores via PE row tiling
                            nc.tensor.matmul(
                                sc[:, 0:512], kt[0:64, jsl], qt[0:64, :],
                                start=True, stop=True,
                            )
                            nc.tensor.matmul(
                                sc[:, 512:1024], kt[64:128, jsl], qt[64:128, :],
                                start=True, stop=True,
                            )
                            ex = wp.tile([128, 1024], BF16, tag="exp", name="ex",
                                         bufs=12)
                            nc.scalar.activation(ex[:], sc[:], Exp, scale=SCALE)
                            exs[jt] = ex
                        if jtt >= LAG:
                            jt = jtt - LAG
                            ex = exs.pop(jt)
                            for hl in range(2):
                                nc.tensor.matmul(
                                    acc[hl][:],
                                    va[jt][:, (2 * p + hl) * VW: (2 * p + hl) * VW + 65],
                                    ex[:, hl * 512:(hl + 1) * 512],
                                    start=(jt == 0),
                                    stop=(jt == N_JT - 1),
                                )
                        # previous block's projection, one l-tile slice at a
                        # time so it never blocks a ready scores matmul for
                        # long on the in-order PE queue
                        if with_proj and ib > 0 and jtt % 2 == 1 and jtt // 2 < 8:
                            proj_unit(ib - 1, (jtt // 2) // 2, (jtt // 2) % 2)
                    if hp:
                        hp.__exit__(None, None, None)
                    for hl in range(2):
                        rc = wp.tile([1, 512], F32, tag="recip", name="rc")
                        nc.vector.reciprocal(rc[:], acc[hl][64:65, :])
                        bc_sb = wp.tile([64, 512], F32, tag="bcast", name="bcsb")
                        nc.gpsimd.partition_broadcast(bc_sb[:], rc[:], channels=64)
                        nc.vector.tensor_mul(
                            attn[p][ib][hl * 64:(hl + 1) * 64, :],
                            acc[hl][0:64, :],
                            bc_sb[:],
                        )
                if with_proj:
                    proj_block(N_IB - 1)

            # emission order = scheduler priority: get pair-0 scores (and so
            # ScalarE exp) started as early as possible
            qk_proj(0)
            v_proj()
            attention(0, with_proj=False)
            qk_proj(1)
            attention(1, with_proj=True)

    nc.compile()
    return nc


_NC_CACHE = None


def _get_nc():
    global _NC_CACHE
    if _NC_CACHE is None:
        _NC_CACHE = build_nc()
    return _NC_CACHE


def _prep_inputs(x, qkv_w, qkv_b):
    """Host-side shard prep -> list of 8 per-core input dicts."""
    bf16 = ml_dtypes.bfloat16
    xT = [np.ascontiguousarray(x[b].T).astype(bf16) for b in range(B)]
    in_maps = []
    for c in range(N_CORES):
        b, hg = divmod(c, 4)
        qr = slice(hg * 256, (hg + 1) * 256)
        kr = slice(D + hg * 256, D + (hg + 1) * 256)
        vr = slice(2 * D + hg * 256, 2 * D + (hg + 1) * 256)
        w_sel = np.concatenate([qkv_w[qr], qkv_w[kr], qkv_w[vr]], axis=0)
        wqkvT = np.ascontiguousarray(w_sel.T).astype(bf16)
        bq = qkv_b[qr].astype(np.float32)
        bk = qkv_b[kr].astype(np.float32)
        bias_qk = np.stack([bq[0:128], bq[128:256], bk[0:128], bk[128:256]], axis=1)
        bias_qk = np.ascontiguousarray(bias_qk)
        in_maps.append({
            "xT": xT[b],
            "wqkvT": wqkvT,
            "bias_qk": bias_qk,
            "projwT": None,  # filled by kernel()
        })
    return in_maps


def kernel(x, qkv_w, qkv_b, proj_w, proj_b):
    x = np.asarray(x, dtype=np.float32)
    qkv_w = np.asarray(qkv_w, dtype=np.float32)
    qkv_b = np.asarray(qkv_b, dtype=np.float32)
    proj_w = np.asarray(proj_w, dtype=np.float32)
    proj_b = np.asarray(proj_b, dtype=np.float32)
    bf16 = ml_dtypes.bfloat16

    in_maps = _prep_inputs(x, qkv_w, qkv_b)
    for c in range(N_CORES):
        hg = c % 4
        pw = proj_w[:, hg * 256:(hg + 1) * 256]  # [D, 256]
        in_maps[c]["projwT"] = np.ascontiguousarray(pw.T).astype(bf16)

    nc = _get_nc()
    res = run_bass_kernel_spmd(nc, in_maps, core_ids=list(range(N_CORES)))

    bv = qkv_b[2 * D: 3 * D]
    bias_full = proj_b + proj_w @ bv  # [D]
    out = np.empty((B, L, D), dtype=np.float32)
    for b in range(B):
        acc = res.results[4 * b]["out"].copy()
        for hg in range(1, 4):
            acc += res.results[4 * b + hg]["out"]
        out[b] = acc + bias_full[None, :]
    return out
